# revision 20
# baseline (speedup 1.0000x reference)
"""Cumulative link (ordinal) loss on 8 Trainium2 NeuronCores.

loss = mean_i [ -ln( sigmoid(hi_i - x_i) - sigmoid(lo_i - x_i) + eps ) ]
with per-label thresholds hi = [0,1,2,3,+inf][l], lo = [-inf,0,1,2,3][l].

Branch-free device formulation (lm = l - 3, H = lm - x, G = H + 3):
    S1 = sigmoid(H + 3)      # = sigmoid(hi - x) when l <= 3   (ACT bias)
    S2 = sigmoid(H + 2)      # = sigmoid(lo - x) when l >= 1   (ACT bias)
    A  = max(lm, S1)         # l==4  ->  1,  else S1
    B  = min(lev, S2)        # l==0  ->  0,  else S2   (lev = lm + 3)
    P  = A - B
    sum ln(P + eps) via ACT Ln with accum_out; host negates / divides.
    All clamps are plain dense-bf16 tensor_tensor ops (DVE 2x mode);
    the scalar_tensor_tensor variant measures 1x and is avoided.

Perf structure (vs the original 3-phase baseline):
  * All DMA on the sync HWDGE ring -- no SWDGE/Q7 descriptor-gen
    startup serialization; the stream starts right after the framework
    preamble and runs at ~430 GB/s (12.6 MB/core in ~29us).
  * Logits stay f32 in SBUF; DVE reads the high bf16 half of each f32
    word (stride-2 view = truncation, ~3e-4 rel on the mean, gate 2e-2).
    Labels' int64 low words are cast once to dense bf16 per tile.
  * DMA granularity 2048 cols (2MB labels + 1MB logits per chunk, near
    line rate); compute granularity 1024 cols for tight pipelining.
  * Ln runs on pair-products P_{2k}*P_{2k+1} (4 passes instead of 8);
    the eps bias absorbs the rare bf16 sigmoid-saturation collisions.
  * ACT program order: sigmoids tiles 0-5, ln pairs 0-2 (table switches
    hidden in DMA-gated ACT slack), sigmoids tiles 6-7, ln pair 3 --
    so the post-DMA tail is one sigmoid pair + one table load + one ln.

Sharding: pure data parallel, 1/8 of batch per core, [128 x 8192].
"""

import numpy as np

B_TOTAL = 8388608
N_CORES = 8
P = 128
SHARD = B_TOTAL // N_CORES          # 1048576 per core
M = SHARD // P                      # 8192 free-dim columns per core
TD = 2048                           # DMA chunk width (columns)
ND = M // TD                        # 4 DMA chunks = 4 ln pairs
T = 1024                            # compute tile width
NT = M // T                         # 8 compute tiles
EPS = 1e-8

_NC = None


def _build_nc():
    import concourse.bacc as bacc
    import concourse.mybir as mybir
    from concourse import tile
    from concourse.tile_rust import add_dep_helper

    f32 = mybir.dt.float32
    bf16 = mybir.dt.bfloat16
    i32 = mybir.dt.int32
    Alu = mybir.AluOpType
    Act = mybir.ActivationFunctionType

    nc = bacc.Bacc("TRN2", target_bir_lowering=False, debug=False,
                   enable_asserts=False)

    x_dram = nc.dram_tensor("logits", (P, M), f32, kind="ExternalInput")
    # int32 pairs at the PJRT boundary (int64 inputs crash the axon run
    # path); low word of each pair is the label value.
    l_dram = nc.dram_tensor("labels", (P, 2 * M), i32, kind="ExternalInput")
    o_dram = nc.dram_tensor("out", (P, ND), f32, kind="ExternalOutput")

    with tile.TileContext(nc) as tc:
        with tc.tile_pool(name="io", bufs=2) as iop, \
             tc.tile_pool(name="work", bufs=4) as wp, \
             tc.psum_pool(name="ps", bufs=4) as psp, \
             tc.tile_pool(name="persist", bufs=1) as pp:
            bias3 = pp.tile([P, 1], f32, tag="bias3")
            nc.vector.memset(bias3[:], 3.0)
            bias2 = pp.tile([P, 1], f32, tag="bias2")
            nc.vector.memset(bias2[:], 2.0)
            # +-identity weights: H = lm - x on the (idle) TensorE, all
            # bf16 so no 4-pass f32 matmuls (that sank an earlier rev).
            ineg = pp.tile([P, P], bf16, tag="ineg")
            nc.vector.memset(ineg[:], 0.0)
            nc.gpsimd.affine_select(out=ineg[:], in_=ineg[:],
                                    compare_op=Alu.not_equal, fill=-1.0,
                                    base=0, pattern=[[-1, P]],
                                    channel_multiplier=1)
            idb = pp.tile([P, P], bf16, tag="idb")
            nc.vector.memset(idb[:], 0.0)
            nc.gpsimd.affine_select(out=idb[:], in_=idb[:],
                                    compare_op=Alu.not_equal, fill=1.0,
                                    base=0, pattern=[[-1, P]],
                                    channel_multiplier=1)
            bias_eps = pp.tile([P, 1], f32, tag="bias_eps")
            nc.vector.memset(bias_eps[:], EPS)
            ppf = pp.tile([P, ND * T], bf16, tag="ppf")   # pair products
            acc = pp.tile([P, ND], f32, tag="acc")

            acts = []           # ACT instructions in intended program order
            p_tiles = [None, None]
            tile_no = [0]

            def chunk(col, width):
                # The first 2048 columns stream as two 1024-col chunks so the
                # DVE pipeline starts ~2.4us earlier; the rest use 2048-col
                # chunks (2MB labels + 1MB logits) for full DMA line rate.
                l32 = iop.tile([P, TD, 2], i32, tag="l32")
                xt = iop.tile([P, TD, 2], bf16, tag="xt")
                nc.sync.dma_start(out=l32[:, :width, :],
                                  in_=l_dram[:, 2 * col:2 * (col + width)])
                nc.sync.dma_start(out=xt[:, :width, :].bitcast(f32),
                                  in_=x_dram[:, col:col + width])
                for s in range(width // T):
                    ls = l32[:, s * T:(s + 1) * T, 0]   # int32 labels, stride 2
                    xs = xt[:, s * T:(s + 1) * T, 1]    # high bf16 of each f32
                    lm = wp.tile([P, T], bf16, tag="lm")
                    lev = wp.tile([P, T], bf16, tag="lev")
                    s1 = wp.tile([P, T], bf16, tag="s1")
                    s2 = wp.tile([P, T], bf16, tag="s2")
                    h = psp.tile([P, T], f32, tag="h")
                    # lm = l - 3  (int32 strided -> dense bf16 cast+shift)
                    nc.vector.tensor_scalar(out=lm[:], in0=ls, scalar1=-3.0,
                                            scalar2=None, op0=Alu.add)
                    # lev = l     (dense single-src, 4x mode)
                    nc.vector.tensor_scalar(out=lev[:], in0=lm[:],
                                            scalar1=3.0, scalar2=None,
                                            op0=Alu.add)
                    # H = lm - x on TensorE (PSUM f32, 512-col banks,
                    # weights grouped so the stationary swaps twice/tile).
                    # The strided bf16 x view feeds the PE directly -- no
                    # cast and no 1x DVE subtract.
                    for k in range(0, T, 512):
                        nc.tensor.matmul(h[:, k:k + 512], ineg[:],
                                         xt[:, s * T + k:s * T + k + 512, 1],
                                         start=True, stop=False)
                    for k in range(0, T, 512):
                        nc.tensor.matmul(h[:, k:k + 512], idb[:],
                                         lm[:, k:k + 512],
                                         start=False, stop=True)
                    acts.append(nc.scalar.activation(s1[:], h[:], Act.Sigmoid,
                                                     bias=bias3[:]))
                    acts.append(nc.scalar.activation(s2[:], h[:], Act.Sigmoid,
                                                     bias=bias2[:]))
                    # A = max(lm, S1) -> s1's slot  (dense 2x, no 1x STT)
                    nc.vector.tensor_tensor(out=s1[:], in0=lm[:], in1=s1[:],
                                            op=Alu.max)
                    # B = min(lev, S2) -> s2's slot
                    nc.vector.tensor_tensor(out=s2[:], in0=lev[:], in1=s2[:],
                                            op=Alu.min)
                    # P = A - B -> s1's slot (A dead after this)
                    nc.vector.tensor_tensor(out=s1[:], in0=s1[:], in1=s2[:],
                                            op=Alu.subtract)
                    t = tile_no[0]
                    tile_no[0] = t + 1
                    p_tiles[t % 2] = s1
                    if t % 2 == 1:
                        d = t // 2
                        nc.vector.tensor_tensor(
                            out=ppf[:, d * T:(d + 1) * T], in0=p_tiles[0][:],
                            in1=p_tiles[1][:], op=Alu.mult)

            def ln_pair(d):
                acts.append(nc.scalar.activation(
                    ppf[:, d * T:(d + 1) * T], ppf[:, d * T:(d + 1) * T],
                    Act.Ln, bias=bias_eps[:], accum_out=acc[:, d:d + 1]))

            chunk(0, T)                 # tiles 0
            chunk(T, T)                 # tile 1
            chunk(2048, 2048)           # tiles 2,3
            chunk(4096, 2048)           # tiles 4,5
            ln_pair(0)
            ln_pair(1)
            ln_pair(2)
            chunk(6144, T)              # tile 6 (split: data lands earlier,
            chunk(7168, T)              # tile 7  shortening the ACT tail)
            ln_pair(3)

            # Freeze the ACT program order exactly as emitted, so the three
            # mid-stream lns (and their table switches) run inside the ACT
            # engine's DMA-gated slack instead of after the last sigmoid.
            for prev, nxt in zip(acts, acts[1:]):
                add_dep_helper(nxt.ins, prev.ins, sync=False,
                               reason="pin ACT order")

            nc.sync.dma_start(out=o_dram[:], in_=acc[:])

    nc.compile()
    return nc


def get_nc():
    global _NC
    if _NC is None:
        _NC = _build_nc()
    return _NC


def make_in_maps(logits, labels):
    x = np.ascontiguousarray(np.asarray(logits, dtype=np.float32)).reshape(B_TOTAL)
    lab = np.asarray(labels)
    if lab.dtype != np.int64:
        lab = lab.astype(np.int64)
    lab = np.ascontiguousarray(lab).reshape(B_TOTAL)
    in_maps = []
    for c in range(N_CORES):
        xs = x[c * SHARD:(c + 1) * SHARD].reshape(P, M)
        ls = lab[c * SHARD:(c + 1) * SHARD].view(np.int32).reshape(P, 2 * M)
        in_maps.append({"logits": xs, "labels": ls})
    return in_maps


def run(logits, labels, trace=False):
    """Returns (loss_scalar_f32, BassKernelResults)."""
    from concourse.bass_utils import run_bass_kernel_spmd

    nc = get_nc()
    in_maps = make_in_maps(logits, labels)
    res = run_bass_kernel_spmd(
        nc, in_maps, core_ids=list(range(N_CORES)), trace=trace
    )
    total = 0.0
    for r in res.results:
        total += r["out"].astype(np.float64).sum()
    loss = np.float32(-total / B_TOTAL)
    return np.asarray(loss), res


def kernel(logits, labels):
    out, _ = run(logits, labels, trace=False)
    return out
